# revision 27
# baseline (speedup 1.0000x reference)
"""ConvCapsuleLayer Trainium2 kernel: 5x5 conv (16->128ch) + 3-iter dynamic routing.

Sharding: H (256) split into 8 bands of 32 rows (halo 2 via host padding).
Each core computes conv + routing for its band; outputs concat along H.

The axon tunnel transfer dominates wall-clock, so the output is shipped as
int8 with per-(row,group) fp32 scales packed into a padding row of the same
output tensor (dequantized on host), and the NC-mean conv input is replaced
by an on-device sum of the per-NC votes. Device exec is cheap relative to
the tunnel, so votes and the routing pipeline are kept in fp32 (row-groups
of 2 rows to fit SBUF), keeping total rel-err (incl. int8 quant) ~6e-3.
Row-broadcasts (c_ij, squash factors, softmax sums) run as fp32 matmuls
against one-hot selectors built on device with affine_select; the DVE
consumes those PSUM tiles directly.
"""
import sys
sys.path.insert(0, "/opt/trn_rl_repo")
import numpy as np

import jax
for _k, _v in (("jax_compilation_cache_dir", "/tmp/jax_comp_cache"),
               ("jax_persistent_cache_min_entry_size_bytes", -1),
               ("jax_persistent_cache_min_compile_time_secs", 0.0)):
    try:
        jax.config.update(_k, _v)
    except Exception:
        pass

import concourse.bass as bass
import concourse.mybir as mybir
import concourse.tile as tile
import concourse.bacc as bacc_mod
from concourse.bass_utils import run_bass_kernel_spmd

dt = mybir.dt
F16 = dt.float16
F32 = dt.float32
I8 = dt.int8
NPF16 = np.float16
AF = mybir.ActivationFunctionType
AX = mybir.AxisListType
EQ = mybir.AluOpType.is_equal

B, NC, LC, H, Wd = 4, 4, 16, 256, 256
NP, LP = 8, 16
NCORES = 8
HB = H // NCORES          # 32 rows per core
RG = 2                    # out-rows per row-group
NG = HB // RG             # 16 row-groups
PIX = RG * Wd             # 512
HPIX = PIX // 2           # 256
WPAD = Wd + 4             # 260

_nc_cache = {}


def build_nc():
    nc = bacc_mod.Bacc()

    xs = nc.declare_dram_parameter("xs", [B, NC, LC, HB + 4, WPAD], F16, isOutput=False)
    wt = nc.declare_dram_parameter("wt", [80, 5, 128], F16, isOutput=False)
    selnp = nc.declare_dram_parameter("selnp", [128, 32], F16, isOutput=False)
    selb = nc.declare_dram_parameter("selb", [128, 4, 32], F32, isOutput=False)
    sumsel = nc.declare_dram_parameter("sumsel", [128, 16], F32, isOutput=False)
    # int8 payload packed into an f32-typed tensor (f32 moves faster through the
    # axon tunnel than int8), channel-major: rows c=0..127 hold quantized data
    # as [c][b][h][w] via bitcast views; row c=128 is exactly the f32 scales.
    y_q = nc.declare_dram_parameter("y_q", [129, B, HB, Wd // 4], F32,
                                    isOutput=True)

    with tile.TileContext(nc) as tc:
        with (
            tc.tile_pool(name="const", bufs=1) as cpool,
            tc.tile_pool(name="xstk", bufs=9) as xpool,
            tc.tile_pool(name="votes", bufs=24) as vpool,
            tc.tile_pool(name="pb", bufs=12) as pbpool,
            tc.tile_pool(name="sqs", bufs=7) as sqpool,
            tc.tile_pool(name="f32w", bufs=10) as fpool,
            tc.tile_pool(name="adds", bufs=8) as apool,
            tc.tile_pool(name="sc", bufs=8) as scpool,
            tc.tile_pool(name="sp1", bufs=2) as sp1,
            tc.tile_pool(name="sp2", bufs=3) as sp2,
            tc.tile_pool(name="vps", bufs=2, space="PSUM") as vps,
            tc.tile_pool(name="ups", bufs=2, space="PSUM") as ups,
            tc.tile_pool(name="cbps", bufs=4, space="PSUM") as cbps,
        ):
            wt_t = cpool.tile([80, 5, 128], F16)
            nc.sync.dma_start(wt_t[:], wt[:])
            selnp_t = cpool.tile([128, 32], F16)
            nc.sync.dma_start(selnp_t[:], selnp[:])
            sumsel_t = cpool.tile([128, 16], F32)
            nc.sync.dma_start(sumsel_t[:], sumsel[:])
            bias_e = cpool.tile([128, 1], F32)
            nc.gpsimd.memset(bias_e[:], 1e-4)
            scal_acc = cpool.tile([128, B * NG], F32)

            # one-hot broadcast selectors, built on device
            # csel[p, (b,n), m] = 1 iff p == n*32 + b*8 + m%8
            csel_t = cpool.tile([128, 16, 128], F32)
            csel_v = csel_t[:].rearrange("p (a b) (c d) -> p a b c d", b=4, d=8)
            nc.gpsimd.memset(csel_v, 1.0)
            nc.gpsimd.affine_select(
                out=csel_v, in_=csel_v, compare_op=EQ, fill=0.0,
                base=0, channel_multiplier=1,
                pattern=[[-8, 4], [-32, 4], [0, 16], [-1, 8]])
            # selb[p, b, j] = 1 iff j == b*8 + p%8  (p%8 not affine -> via csel rows)
            # fsel[p, (n,b,np)] = 1 iff p == 32*b + np
            fsel_t = cpool.tile([128, 128], F32)
            fsel_v = fsel_t[:].rearrange("p (a b c) -> p a b c", b=4, c=8)
            nc.gpsimd.memset(fsel_v, 1.0)
            nc.gpsimd.affine_select(
                out=fsel_v, in_=fsel_v, compare_op=EQ, fill=0.0,
                base=0, channel_multiplier=1,
                pattern=[[0, 4], [-32, 4], [-1, 8]])
            # rsel[p, q] = 1 iff p == q//8
            rsel_t = cpool.tile([16, 128], F32)
            rsel_v = rsel_t[:].rearrange("p (a b) -> p a b", b=8)
            nc.gpsimd.memset(rsel_v, 1.0)
            nc.gpsimd.affine_select(
                out=rsel_v, in_=rsel_v, compare_op=EQ, fill=0.0,
                base=0, channel_multiplier=1, pattern=[[-1, 16], [0, 8]])
            # fbsel[p, b, m] = 1 iff p == 32*b + m%8
            fbsel_t = cpool.tile([128, 4, 128], F32)
            fbsel_v = fbsel_t[:].rearrange("p a (c d) -> p a c d", d=8)
            nc.gpsimd.memset(fbsel_v, 1.0)
            nc.gpsimd.affine_select(
                out=fbsel_v, in_=fbsel_v, compare_op=EQ, fill=0.0,
                base=0, channel_multiplier=1,
                pattern=[[-32, 4], [0, 16], [-1, 8]])
            # selb[p, b, j] = 1 iff j == b*8 + p%8 depends on p%8 in a column
            # index (not affine in p), so it is shipped like selnp/sumsel.
            selb_t = cpool.tile([128, 4, 32], F32)
            nc.sync.dma_start(selb_t[:], selb[:])

            for g in range(NG):
                s0 = g * RG
                votes = {}
                pb16 = {}
                sqs = {}
                for b in range(B):
                    stk = []
                    for n in range(NC):
                        t = xpool.tile([80, RG, WPAD], F16, tag="xstk")
                        src = xs[b, n, :, s0: s0 + RG, :]
                        src.ap = [[WPAD, 5]] + src.ap   # overlapping ky dim
                        nc.sync.dma_start(t[:], src)
                        stk.append(t)

                    for n in range(NC):
                        vt = vpool.tile([128, PIX], F32, tag="votes")
                        ph = vps.tile([128, PIX], F32, tag="vps",
                                      name=f"vps{g}_{b}_{n}")
                        for kx in range(5):
                            nc.tensor.matmul(
                                ph[:], wt_t[:, kx, :],
                                stk[n][:, :, kx: kx + Wd],
                                start=(kx == 0), stop=(kx == 4))
                        nc.scalar.copy(vt[:], ph[:])
                        votes[(b, n)] = vt
                    # iter-0 parent_bs = sum_n votes / 8 (uniform c_ij), plus its square
                    s01 = apool.tile([128, PIX], F32, tag="adds")
                    s23 = apool.tile([128, PIX], F32, tag="adds")
                    ssum = apool.tile([128, PIX], F32, tag="adds")
                    nc.vector.tensor_add(s01[:], votes[(b, 0)][:], votes[(b, 1)][:])
                    nc.vector.tensor_add(s23[:], votes[(b, 2)][:], votes[(b, 3)][:])
                    nc.vector.tensor_add(ssum[:], s01[:], s23[:])
                    v0 = pbpool.tile([128, PIX], F32, tag="pb")
                    sq0 = sqpool.tile([128, PIX], F16, tag="sqs")
                    nc.scalar.mul(v0[:], ssum[:], 0.125)
                    nc.scalar.activation(sq0[:], ssum[:], AF.Square, scale=0.125)
                    pb16[b] = v0
                    sqs[b] = sq0

                sims = sp2.tile([128, PIX], F32, tag="sims")

                for it in range(3):
                    if it > 0:
                        for b in range(B):
                            sq = sqpool.tile([128, PIX], F16, tag="sqs")
                            nc.vector.tensor_mul(sq[:], pb16[b][:], pb16[b][:])
                            sqs[b] = sq
                    # sq_all rows b*32+np via col-tiled selector mms
                    sqp = cbps.tile([128, PIX], F32, tag="cbps", name=f"sq{g}_{it}")
                    for b in range(B):
                        nc.tensor.matmul(
                            sqp[32 * b:32 * (b + 1), :], selnp_t[:],
                            sqs[b][:], start=True, stop=True,
                            tile_position=(0, 32 * b))
                    sr = sp1.tile([128, PIX], F32, tag="sr")
                    dd = sp1.tile([128, PIX], F32, tag="dd")
                    nc.scalar.activation(sr[:], sqp[:], AF.Sqrt)
                    nc.vector.tensor_scalar_add(dd[:], sqp[:], 1.0 + 1e-4)
                    rd = sp1.tile([128, PIX], F32, tag="rd")
                    nc.vector.reciprocal_approx_fast(rd[:], dd[:])
                    fac = sp2.tile([128, PIX], F32, tag="fac")
                    nc.vector.tensor_mul(fac[:], sr[:], rd[:])

                    if it < 2:
                        uh = ups.tile([128, PIX], F32, tag="ups", name=f"uh{it}")
                        for b in range(B):
                            for n in range(NC):
                                r = fpool.tile([128, PIX], F32, tag="f32w")
                                nc.vector.tensor_mul(r[:], votes[(b, n)][:], pb16[b][:])
                                nc.tensor.matmul(
                                    uh[32 * n:32 * (n + 1), :],
                                    selb_t[:, b, :], r[:],
                                    start=(b == 0), stop=(b == B - 1),
                                    tile_position=(0, 32 * n))
                        tgt = sims if it == 0 else sp2.tile([128, PIX], F32, tag="fu", name=f"fu{it}")
                        # DVE reads at most one PSUM operand; stage uh in SBUF
                        uhs = sp1.tile([128, PIX], F32, tag=f"uhs{it}")
                        nc.scalar.copy(uhs[:], uh[:])
                        fps = cbps.tile([128, PIX], F32, tag="cbps",
                                        name=f"facr{g}_{it}")
                        nc.tensor.matmul(fps[:], fsel_t[:], fac[:],
                                         start=True, stop=True)
                        nc.vector.tensor_mul(tgt[:], fps[:], uhs[:])
                        if it > 0:
                            nc.vector.tensor_add(sims[:], sims[:], tgt[:])

                        e = sp1.tile([128, PIX], F32, tag="e")
                        nc.scalar.activation(e[:], sims[:], AF.Exp, bias=bias_e[:])
                        rs = sp2.tile([16, PIX], F32, tag="rs")
                        call = sp2.tile([128, PIX], F32, tag="call")
                        sp_ = cbps.tile([16, PIX], F32, tag="cbps",
                                        name=f"se{g}_{it}")
                        nc.tensor.matmul(sp_[:], sumsel_t[:], e[:],
                                         start=True, stop=True)
                        nc.vector.reciprocal_approx_fast(rs[:], sp_[:])
                        rps = cbps.tile([128, PIX], F32, tag="cbps",
                                        name=f"rsb{g}_{it}")
                        nc.tensor.matmul(rps[:], rsel_t[:], rs[:],
                                         start=True, stop=True)
                        nc.vector.tensor_mul(call[:], e[:], rps[:])

                        for b in range(B):
                            pb = pbpool.tile([128, PIX], F32, tag="pb")
                            t1 = apool.tile([128, PIX], F32, tag="adds")
                            t2 = apool.tile([128, PIX], F32, tag="adds")
                            prev_q = None
                            for n in range(NC):
                                q = fpool.tile([128, PIX], F32, tag="f32w")
                                cps = cbps.tile([128, PIX], F32, tag="cbps",
                                                name=f"cb{g}_{it}_{b}_{n}")
                                nc.tensor.matmul(cps[:], csel_t[:, b * 4 + n, :],
                                                 call[:], start=True, stop=True)
                                nc.vector.tensor_mul(q[:], cps[:], votes[(b, n)][:])
                                if n == 1:
                                    nc.vector.tensor_add(t1[:], prev_q[:], q[:])
                                elif n == 3:
                                    nc.vector.tensor_add(t2[:], prev_q[:], q[:])
                                prev_q = q
                            nc.vector.tensor_add(pb[:], t1[:], t2[:])
                            pb16[b] = pb
                    else:
                        for b in range(B):
                            out = sp2.tile([128, PIX], F32, tag="outt")
                            fps = cbps.tile([128, PIX], F32, tag="cbps",
                                            name=f"fb{g}_{b}")
                            nc.tensor.matmul(fps[:], fbsel_t[:, b, :],
                                             fac[:], start=True, stop=True)
                            nc.vector.tensor_mul(out[:], fps[:], pb16[b][:])
                            # int8 quantization: per-partition absmax -> scale
                            amax = scpool.tile([128, 1], F32, tag="sc")
                            nc.vector.reduce_max(amax[:], out[:], axis=AX.X,
                                                 apply_absolute_value=True)
                            nc.vector.tensor_scalar_max(amax[:], amax[:], 1e-20)
                            rcp = scpool.tile([128, 1], F32, tag="sc")
                            nc.vector.reciprocal(rcp[:], amax[:])
                            scl = scpool.tile([128, 1], F32, tag="sc")
                            nc.vector.tensor_scalar_mul(scl[:], rcp[:], 127.0)
                            outq = sp2.tile([128, PIX], I8, tag="outq")
                            nc.scalar.activation(outq[:], out[:], AF.Copy, scale=scl[:])
                            nc.sync.dma_start(
                                y_q[0:128, b, s0:s0 + RG, :].bitcast(I8).rearrange(
                                    "(p l) r w -> l p r w", p=8, l=16),
                                outq.rearrange("p (r w) -> p r w", r=RG))
                            col = b * NG + g
                            nc.vector.tensor_scalar_mul(
                                scal_acc[:, col:col + 1], amax[:], 1.0 / 127.0)
            nc.sync.dma_start(y_q[128].rearrange("b h w -> (b h) w"), scal_acc[:])

    nc.compile()
    return nc


def _prep_inputs(x, W):
    x = np.asarray(x, np.float32)
    W = np.asarray(W, np.float32)
    # oc' = lp*8+np ordering of output channels
    perm = np.zeros(128, np.int64)
    for np_ in range(8):
        for lp in range(16):
            perm[lp * 8 + np_] = np_ * 16 + lp
    wt = np.zeros((80, 5, 128), np.float32)
    for kx in range(5):
        for ky in range(5):
            wt[ky * 16:(ky + 1) * 16, kx, :] = W[perm, :, ky, kx].T
    wt = wt.astype(NPF16)

    selnp = np.zeros((128, 32), NPF16)
    for p in range(128):
        selnp[p, p % 8] = 1.0
    selb = np.zeros((128, 4, 32), np.float32)
    for b in range(4):
        for p in range(128):
            selb[p, b, b * 8 + p % 8] = 1.0
    sumsel = np.zeros((128, 16), np.float32)
    for p in range(128):
        sumsel[p, (p // 32) * 4 + (p % 32) // 8] = 1.0

    xp16 = np.zeros((B, NC, LC, H + 4, WPAD), NPF16)
    xp16[:, :, :, 2:-2, 2:-2] = x

    in_maps = []
    for k in range(NCORES):
        r0 = k * HB
        in_maps.append({
            "xs": np.ascontiguousarray(xp16[:, :, :, r0:r0 + HB + 4, :]),
            "wt": wt, "selnp": selnp, "selb": selb, "sumsel": sumsel,
        })
    return in_maps


# channel c = np*16+lp  ->  sbuf row m = lp*8+np
_MINV = np.array([(c % 16) * 8 + c // 16 for c in range(128)], np.int64)


def kernel(x, W):
    if "nc" not in _nc_cache:
        _nc_cache["nc"] = build_nc()
    nc = _nc_cache["nc"]
    in_maps = _prep_inputs(x, W)
    res = None
    for attempt in range(3):
        try:
            res = run_bass_kernel_spmd(nc, in_maps, list(range(NCORES))).results
            break
        except Exception:
            # transient NRT_EXEC_UNIT_UNRECOVERABLE wedges recover on rerun
            if attempt == 2:
                raise
    parts = []
    for r in res:
        buf = r["y_q"]                                       # [129,B,HB,Wd/4] f32
        by = buf[:128].view(np.int8)                         # [128c,B,HB,Wd]
        q = by.transpose(1, 0, 2, 3).astype(np.float32).reshape(B, 128, NG, RG, Wd)
        sb = buf[128].reshape(128, B * NG)                   # [128,B*NG] f32
        s = sb.reshape(128, B, NG).transpose(1, 0, 2)        # [B,128m,NG]
        s = s[:, _MINV, :]                                   # [B,128c,NG]
        parts.append((q * s[:, :, :, None, None]).reshape(B, 128, HB, Wd))
    out = np.concatenate(parts, axis=2)
    return out.reshape(B, NP, LP, H, Wd)
